# revision 4
# baseline (speedup 1.0000x reference)
"""DeepseekV3 MoE "calibrate-all-experts" kernel for 8 Trainium2 NeuronCores.

Sharding: expert-parallel. Each core owns E/8 routed experts plus a 1/8
slice of the shared-expert intermediate dim. Tokens are replicated; each
core computes its partial (weighted routed sum + shared-expert partial)
over all tokens. Per-(token-chunk, H-half) ReduceScatters combine
partials while scattering (overlapping compute), and the host
reassembles the shards.

Structure tuned for PE-array continuity on TRN2 (pstate ramps punish
idle gaps; LDWEIGHTS serializes between matmuls):
  - the fp32 router for ALL token chunks runs up front, so the bf16
    expert phases form one long uninterrupted matmul stream
  - token chunks of 1024: every stationary tile feeds two 512-wide
    moving matmuls back-to-back (walrus can skip the second LDWEIGHTS)
  - down-projection keeps the down-weights stationary; output lands
    [H, tokens]-major and the host transposes
  - per-expert gate weights folded into the activations before the
    down-projection so all experts + the shared expert accumulate into
    a single PSUM group per (h tile, token half)
"""
import sys

if '/opt/trn_rl_repo' not in sys.path:
    sys.path.insert(0, '/opt/trn_rl_repo')

import numpy as np
import ml_dtypes

import concourse.bass as bass
import concourse.mybir as mybir
import concourse.tile as tile
from concourse import bacc
from concourse.bass import ds, ts
from concourse.bass_utils import run_bass_kernel_spmd
from concourse.masks import make_identity

F32 = mybir.dt.float32
BF16 = mybir.dt.bfloat16
P = 128

FULL = dict(T=2048, H=2048, E=32, I=1024, IS=2048, n_cores=8)
ROUTED_SCALING = 2.5


def build_moe_nc(T, H, E, I, IS, n_cores, TC=None):
    E_LOC = E // n_cores
    IS_LOC = IS // n_cores
    KH = H // P            # contraction tiles over H
    KI = I // P            # expert intermediate tiles
    KIS = IS_LOC // P      # shared intermediate tiles (per core)
    if TC is None:
        TC = min(1024, T)  # token chunk (two 512-wide moving halves)
    NCH = T // TC          # number of token chunks
    TH = TC // 2           # moving half width
    TCR = 256              # router sub-chunk
    NHH = 2                # H halves for the ReduceScatter
    HH = H // NHH
    assert H % P == 0 and I % P == 0 and IS_LOC % P == 0 and T % TC == 0

    nc = bacc.Bacc("TRN2", target_bir_lowering=False, debug=False,
                   num_devices=n_cores)

    xT32 = nc.dram_tensor("xT32", [H, T], F32, kind="ExternalInput")
    xTb = nc.dram_tensor("xTb", [H, T], BF16, kind="ExternalInput")
    gwT = nc.dram_tensor("gwT", [H, E], F32, kind="ExternalInput")
    wg = nc.dram_tensor("wg", [E_LOC, H, I], BF16, kind="ExternalInput")
    wu = nc.dram_tensor("wu", [E_LOC, H, I], BF16, kind="ExternalInput")
    wd = nc.dram_tensor("wd", [E_LOC, I, H], BF16, kind="ExternalInput")
    wsg = nc.dram_tensor("wsg", [H, IS_LOC], BF16, kind="ExternalInput")
    wsu = nc.dram_tensor("wsu", [H, IS_LOC], BF16, kind="ExternalInput")
    wsd = nc.dram_tensor("wsd", [IS_LOC, H], BF16, kind="ExternalInput")
    # shards: [chunk, h_half, h rows, TC tokens] flattened
    out_shard = nc.dram_tensor(
        "out_shard", [NCH * NHH * (HH // n_cores), TC], BF16,
        kind="ExternalOutput")

    xT32_t = xT32.ap().rearrange("(ko p) t -> p ko t", p=P)
    xTb_t = xTb.ap().rearrange("(ko p) t -> p ko t", p=P)
    gwT_t = gwT.ap().rearrange("(ko p) e -> p ko e", p=P)

    with tile.TileContext(nc) as tc:
        with (
            tc.tile_pool(name="cpool", bufs=1) as cpool,
            tc.tile_pool(name="dram", bufs=1, space="DRAM") as dram,
            tc.tile_pool(name="xfpool", bufs=2) as xfpool,
            tc.tile_pool(name="xpool", bufs=1) as xpool,
            tc.tile_pool(name="rsc", bufs=3) as rsc,
            tc.tile_pool(name="wpool", bufs=2) as wpool,
            tc.tile_pool(name="spool", bufs=2) as spool,
            tc.tile_pool(name="apool", bufs=E_LOC * KI + KIS + 1) as apool,
            tc.tile_pool(name="wbpool", bufs=E_LOC + 1) as wbpool,
            tc.tile_pool(name="dpool", bufs=6) as dpool,
            tc.tile_pool(name="opool", bufs=2) as opool,
            tc.tile_pool(name="pgp", bufs=4, space="PSUM") as pgp,
            tc.tile_pool(name="pop", bufs=4, space="PSUM") as pop,
        ):
            ident = cpool.tile([P, P], F32)
            make_identity(nc, ident[:])
            gw_sb = cpool.tile([P, KH, E], F32)
            nc.sync.dma_start(gw_sb[:], gwT_t)

            # dense routing weights of the local experts, transposed [E_LOC, T]
            wT_dram = dram.tile([E_LOC, T], BF16)
            partials = [dram.tile([H, TC], BF16, name=f"partial_{c4}")
                        for c4 in range(NCH)]
            rs_outs = [dram.tile([HH // n_cores, TC], BF16,
                                 name=f"rsout_{c4}_{hh}")
                       for c4 in range(NCH) for hh in range(NHH)]

            # ---------- router for ALL chunks (fp32), up front ----------
            for r4 in range(T // TCR):
                xf = xfpool.tile([P, KH, TCR], F32, tag="xf")
                nc.sync.dma_start(xf[:], xT32_t[:, :, ds(r4 * TCR, TCR)])
                for t in range(TCR // P):
                    zp = pgp.tile([P, E], F32, tag="pg")
                    for k in range(KH):
                        nc.tensor.matmul(zp[:], xf[:, k, ts(t, P)],
                                         gw_sb[:, k, :],
                                         start=(k == 0), stop=(k == KH - 1))
                    s_sb = rsc.tile([P, E], F32, tag="s")
                    nc.scalar.activation(
                        s_sb[:], zp[:], mybir.ActivationFunctionType.Sigmoid)
                    z_sb = rsc.tile([P, E], F32, tag="zs")
                    nc.vector.tensor_copy(z_sb[:], zp[:])
                    top8 = rsc.tile([P, 8], F32, tag="t8")
                    nc.vector.max(top8[:], z_sb[:])
                    dw = rsc.tile([P, E], F32, tag="dw")
                    # mask of selected experts: z >= (8th largest z)
                    nc.vector.tensor_scalar(
                        dw[:], z_sb[:], top8[:, 7:8], None,
                        op0=mybir.AluOpType.is_ge)
                    # masked sigmoid scores
                    nc.vector.tensor_mul(dw[:], s_sb[:], dw[:])
                    ssum = rsc.tile([P, 1], F32, tag="ss")
                    nc.vector.reduce_sum(ssum[:], dw[:],
                                         axis=mybir.AxisListType.X)
                    nc.vector.tensor_scalar_add(ssum[:], ssum[:], 1e-20)
                    inv = rsc.tile([P, 1], F32, tag="iv")
                    nc.vector.reciprocal(inv[:], ssum[:])
                    # dense weights = masked_s / sum * ROUTED_SCALING
                    nc.vector.tensor_scalar(
                        dw[:], dw[:], inv[:], float(ROUTED_SCALING),
                        op0=mybir.AluOpType.mult, op1=mybir.AluOpType.mult)
                    # transpose the local experts' columns -> [E_LOC, P]
                    tp = pgp.tile([P, P], F32, tag="pg")
                    nc.tensor.transpose(tp[:E_LOC, :], dw[:, :E_LOC],
                                        ident[:])
                    wtt = rsc.tile([E_LOC, P], BF16, tag="wtt")
                    nc.vector.tensor_copy(wtt[:], tp[:E_LOC, :])
                    nc.sync.dma_start(
                        wT_dram[:, ds(r4 * TCR + t * P, P)], wtt[:])

            for c4 in range(NCH):
                xb = xpool.tile([P, KH, TC], BF16, tag="xb")
                nc.sync.dma_start(xb[:], xTb_t[:, :, ds(c4 * TC, TC)])

                # broadcast this chunk's routing weights of the local
                # experts across all partitions (0-stride partition DMA)
                Wsb = []
                for e in range(E_LOC):
                    w_e = wbpool.tile([P, TC], BF16, tag="W")
                    nc.sync.dma_start(
                        w_e[:],
                        wT_dram[e, ds(c4 * TC, TC)].partition_broadcast(P))
                    Wsb.append(w_e)

                # ---- phase 1: gate/up projections + silu(g)*u*(gate wt)
                # weights in I-halves; each stationary tile feeds both
                # moving token halves back-to-back
                acts = {}
                for e in range(E_LOC):
                    for ih in range(2):
                        isl = ds(ih * (I // 2), I // 2)
                        wg_sb = wpool.tile([P, KH, I // 2], BF16, tag="w")
                        nc.sync.dma_start(
                            wg_sb[:], wg.ap()[e][:, isl].rearrange(
                                "(ko p) i -> p ko i", p=P))
                        wu_sb = wpool.tile([P, KH, I // 2], BF16, tag="w")
                        nc.sync.dma_start(
                            wu_sb[:], wu.ap()[e][:, isl].rearrange(
                                "(ko p) i -> p ko i", p=P))
                        for i in range(KI // 2):
                            pg = [pgp.tile([P, TH], F32, tag="pg",
                                            name=f"pg_{e}_{ih}_{i}_{th2}")
                                  for th2 in range(2)]
                            for k in range(KH):
                                for th in range(2):
                                    nc.tensor.matmul(
                                        pg[th][:], wg_sb[:, k, ts(i, P)],
                                        xb[:, k, ds(th * TH, TH)],
                                        start=(k == 0), stop=(k == KH - 1))
                            sg = spool.tile([P, TC], F32, tag="sg")
                            for th in range(2):
                                nc.scalar.activation(
                                    sg[:, ds(th * TH, TH)], pg[th][:],
                                    mybir.ActivationFunctionType.Sigmoid)
                                nc.vector.tensor_mul(
                                    sg[:, ds(th * TH, TH)],
                                    sg[:, ds(th * TH, TH)], pg[th][:])
                            pu = [pgp.tile([P, TH], F32, tag="pg",
                                            name=f"pu_{e}_{ih}_{i}_{th2}")
                                  for th2 in range(2)]
                            for k in range(KH):
                                for th in range(2):
                                    nc.tensor.matmul(
                                        pu[th][:], wu_sb[:, k, ts(i, P)],
                                        xb[:, k, ds(th * TH, TH)],
                                        start=(k == 0), stop=(k == KH - 1))
                            a = apool.tile([P, TC], BF16, tag="act")
                            for th in range(2):
                                nc.vector.tensor_mul(
                                    a[:, ds(th * TH, TH)],
                                    sg[:, ds(th * TH, TH)], pu[th][:])
                            nc.vector.tensor_mul(a[:], a[:], Wsb[e][:])
                            acts[(e, ih * (KI // 2) + i)] = a

                # ---- shared expert (gate weight is 1)
                wsg_sb = wpool.tile([P, KH, IS_LOC], BF16, tag="w")
                nc.sync.dma_start(
                    wsg_sb[:], wsg.ap().rearrange("(ko p) i -> p ko i", p=P))
                wsu_sb = wpool.tile([P, KH, IS_LOC], BF16, tag="w")
                nc.sync.dma_start(
                    wsu_sb[:], wsu.ap().rearrange("(ko p) i -> p ko i", p=P))
                for i in range(KIS):
                    pg = [pgp.tile([P, TH], F32, tag="pg", name=f"spg_{i}_{th2}")
                          for th2 in range(2)]
                    for k in range(KH):
                        for th in range(2):
                            nc.tensor.matmul(
                                pg[th][:], wsg_sb[:, k, ts(i, P)],
                                xb[:, k, ds(th * TH, TH)],
                                start=(k == 0), stop=(k == KH - 1))
                    sg = spool.tile([P, TC], F32, tag="sg")
                    for th in range(2):
                        nc.scalar.activation(
                            sg[:, ds(th * TH, TH)], pg[th][:],
                            mybir.ActivationFunctionType.Sigmoid)
                        nc.vector.tensor_mul(
                            sg[:, ds(th * TH, TH)],
                            sg[:, ds(th * TH, TH)], pg[th][:])
                    pu = [pgp.tile([P, TH], F32, tag="pg", name=f"spu_{i}_{th2}")
                          for th2 in range(2)]
                    for k in range(KH):
                        for th in range(2):
                            nc.tensor.matmul(
                                pu[th][:], wsu_sb[:, k, ts(i, P)],
                                xb[:, k, ds(th * TH, TH)],
                                start=(k == 0), stop=(k == KH - 1))
                    a = apool.tile([P, TC], BF16, tag="act")
                    for th in range(2):
                        nc.vector.tensor_mul(
                            a[:, ds(th * TH, TH)],
                            sg[:, ds(th * TH, TH)], pu[th][:])
                    acts[("s", i)] = a

                # ---- phase 2: down-projections, wd stationary, reused
                # across both token halves. Output is [H, tokens]-major;
                # all experts + shared accumulate into one PSUM group per
                # (h tile, token half).
                n_k = E_LOC * KI + KIS
                for hh in range(NHH):
                    for hq in range(HH // P):
                        hrow = hh * HH + hq * P
                        wd_tiles = []
                        for e in range(E_LOC):
                            wdt = dpool.tile([P, KI, P], BF16, tag="wd")
                            nc.sync.dma_start(
                                wdt[:],
                                wd.ap()[e][:, ds(hrow, P)].rearrange(
                                    "(i p) h -> p i h", p=P))
                            wd_tiles.append(wdt)
                        wsdt = dpool.tile([P, KIS, P], BF16, tag="wsd")
                        nc.sync.dma_start(
                            wsdt[:],
                            wsd.ap()[:, ds(hrow, P)].rearrange(
                                "(i p) h -> p i h", p=P))
                        po = [pop.tile([P, TH], F32, tag="po",
                                        name=f"po_{c4}_{hh}_{hq}_{th2}")
                              for th2 in range(2)]
                        kidx = 0
                        for e in range(E_LOC):
                            for i in range(KI):
                                for th in range(2):
                                    nc.tensor.matmul(
                                        po[th][:],
                                        wd_tiles[e][:, i, :],
                                        acts[(e, i)][:, ds(th * TH, TH)],
                                        start=(kidx == 0),
                                        stop=(kidx == n_k - 1))
                                kidx += 1
                        for i in range(KIS):
                            for th in range(2):
                                nc.tensor.matmul(
                                    po[th][:],
                                    wsdt[:, i, :],
                                    acts[("s", i)][:, ds(th * TH, TH)],
                                    start=(kidx == 0),
                                    stop=(kidx == n_k - 1))
                            kidx += 1
                        ost = opool.tile([P, TC], BF16, tag="ost")
                        for th in range(2):
                            nc.vector.tensor_copy(
                                ost[:, ds(th * TH, TH)], po[th][:])
                        nc.sync.dma_start(
                            partials[c4][ds(hrow, P), :], ost[:])

                    # ---- combine this (chunk, h half) across cores
                    rs_o = rs_outs[c4 * NHH + hh]
                    if n_cores > 1:
                        nc.gpsimd.collective_compute(
                            "ReduceScatter",
                            mybir.AluOpType.add,
                            ins=[partials[c4][ds(hh * HH, HH), :].opt()],
                            outs=[rs_o.opt()],
                            replica_groups=[list(range(n_cores))],
                        )
                        nc.sync.dma_start(
                            out_shard.ap()[
                                ds((c4 * NHH + hh) * (HH // n_cores),
                                   HH // n_cores), :],
                            rs_o[:])
                    else:
                        nc.sync.dma_start(
                            out_shard.ap()[
                                ds((c4 * NHH + hh) * HH, HH), :],
                            partials[c4][ds(hh * HH, HH), :])

    nc.compile()
    return nc


def make_in_maps(hidden_states, gate_weight, w_gate, w_up, w_down,
                 ws_gate, ws_up, ws_down, n_cores):
    """Host-side shard/layout prep (pure data movement + dtype casts)."""
    B, S, H = hidden_states.shape
    T = B * S
    E = gate_weight.shape[0]
    IS = ws_gate.shape[1]
    E_LOC = E // n_cores
    IS_LOC = IS // n_cores
    bf16 = ml_dtypes.bfloat16

    x = np.asarray(hidden_states, dtype=np.float32).reshape(T, H)
    xT32 = np.ascontiguousarray(x.T)
    xTb = xT32.astype(bf16)

    in_maps = []
    for c in range(n_cores):
        loc = list(range(c * E_LOC, (c + 1) * E_LOC))
        rest = [e for e in range(E) if e not in loc]
        perm = loc + rest
        gwT_c = np.ascontiguousarray(
            np.asarray(gate_weight, np.float32)[perm].T)
        in_maps.append({
            "xT32": xT32,
            "xTb": xTb,
            "gwT": gwT_c,
            "wg": np.ascontiguousarray(w_gate[loc]).astype(bf16),
            "wu": np.ascontiguousarray(w_up[loc]).astype(bf16),
            "wd": np.ascontiguousarray(w_down[loc]).astype(bf16),
            "wsg": np.ascontiguousarray(
                ws_gate[:, c * IS_LOC:(c + 1) * IS_LOC]).astype(bf16),
            "wsu": np.ascontiguousarray(
                ws_up[:, c * IS_LOC:(c + 1) * IS_LOC]).astype(bf16),
            "wsd": np.ascontiguousarray(
                ws_down[c * IS_LOC:(c + 1) * IS_LOC, :]).astype(bf16),
        })
    return in_maps


def assemble_output(results, T, H, n_cores, TC):
    """Un-interleave the per-(chunk, h-half) ReduceScatter shards.

    Shard row layout: (chunk c4, h half hh, h rows); each row holds TC
    tokens ([H, T]-major on device -> transpose per block here).
    """
    NCH = T // TC
    NHH = 2
    HH = H // NHH
    rows = HH // n_cores  # h rows per (chunk, half) shard
    out = np.empty((T, H), np.float32)
    for r in range(n_cores):
        res_r = results[r]["out_shard"].astype(np.float32)
        for c4 in range(NCH):
            for hh in range(NHH):
                blk = res_r[(c4 * NHH + hh) * rows:
                            (c4 * NHH + hh + 1) * rows]  # [rows, TC]
                h0 = hh * HH + r * rows
                out[c4 * TC:(c4 + 1) * TC, h0:h0 + rows] = blk.T
    return out


_NC_CACHE = None


def _get_nc():
    global _NC_CACHE
    if _NC_CACHE is None:
        _NC_CACHE = build_moe_nc(**FULL)
    return _NC_CACHE


def kernel(hidden_states, gate_weight, w_gate, w_up, w_down,
           ws_gate, ws_up, ws_down):
    B, S, H = hidden_states.shape
    T = B * S
    n_cores = FULL["n_cores"]
    TC = min(1024, T)
    in_maps = make_in_maps(hidden_states, gate_weight, w_gate, w_up, w_down,
                           ws_gate, ws_up, ws_down, n_cores)
    nc = _get_nc()
    res = run_bass_kernel_spmd(nc, in_maps, core_ids=list(range(n_cores)))
    out = assemble_output(res.results, T, H, n_cores, TC)
    return np.ascontiguousarray(
        out.reshape(B, S, H).astype(np.asarray(hidden_states).dtype))
